# revision 23
# baseline (speedup 1.0000x reference)
"""MoE (BailingMoeV2.5) Trainium2 kernel — 8-core expert-parallel.

Problem: T=2048 tokens, H=2048 hidden, E=16 experts (groups of 4, top-2
groups, top-4 experts), I=1024 expert intermediate, shared expert IS=1024,
routed scale 2.5.

Sharding: core c owns experts {2c, 2c+1}. Each core:
  1. fp32 router (replicated): grouped top-k -> dense combine matrix C[T,16]
     (renormalized raw-sigmoid weights * 2.5, zeros elsewhere).
  2. fp32r dense-masked expert FFN for its 2 experts:
       y_e = silu(x @ w1_e.T) * (x @ w3_e.T)   (feature-major, via DRAM)
       z   = sum_e C[:,e] * (y_e @ w2_e.T)     (token-major)
  3. ReduceScatter(add) of z over the 8 cores -> [256, 2048] token slice.
  4. Shared expert (fp32r) on its 256-token slice, added to the RS result.
Host concatenates the 8 [256, 2048] outputs.

Everything data-dependent is identical across cores; per-core behavior comes
only from per-core input tensors (weight slices + shared-expert token slice).
"""
import os
import sys

sys.path.insert(0, "/opt/trn_rl_repo")

import numpy as np

import concourse.bass as bass
import concourse.mybir as mybir
import concourse.tile as tile
from concourse import bacc
from concourse.bass_utils import run_bass_kernel_spmd
from concourse.masks import make_identity

P = 128
T, H, E, K_TOP, I = 2048, 2048, 16, 4, 1024
G = 4                      # expert groups
IS = 1024                  # shared-expert intermediate
N_CORES = 8
E_PER_CORE = E // N_CORES  # 2
TS = T // N_CORES          # 256 token slice per core for shared/final
ROUTED_SCALE = 2.5

KT_H = H // P              # 16 k-tiles over H
KT_I = I // P              # 8 k-tiles over I
NTOK = 8                   # token chunks of 256 for stage A
TCH = T // NTOK            # 512
TT = T // P                # 16 token tiles of 128
HC = H // 512              # 4 output column chunks of 512
IH = 2                     # I halves for stage A weight residency
IHW = I // IH              # 512

F32 = mybir.dt.float32
F32R = mybir.dt.float32r
AX = mybir.AxisListType.X
ALU = mybir.AluOpType
AF = mybir.ActivationFunctionType


def _r3(ap, p=P):
    """DRAM [K, N] -> [P, K//P, N] k-tile view."""
    return ap.rearrange("(kt p) n -> p kt n", p=p)


def build_nc():
    nc = bacc.Bacc(None, target_bir_lowering=False, debug=False)

    # ---- per-core inputs (fp32r unless noted) ----
    xT_d = nc.declare_dram_parameter("xT", [H, T], F32R, isOutput=False)
    gwT_d = nc.declare_dram_parameter("gwT", [H, E], F32, isOutput=False)
    biasb_d = nc.declare_dram_parameter("biasb", [P, E], F32, isOutput=False)
    w1t_d = nc.declare_dram_parameter("w1t", [E_PER_CORE, H, I], F32R, isOutput=False)
    w3t_d = nc.declare_dram_parameter("w3t", [E_PER_CORE, H, I], F32R, isOutput=False)
    w2t_d = nc.declare_dram_parameter("w2t", [E_PER_CORE, I, H], F32R, isOutput=False)
    sw1t_d = nc.declare_dram_parameter("sw1t", [H, IS], F32R, isOutput=False)
    sw3t_d = nc.declare_dram_parameter("sw3t", [H, IS], F32R, isOutput=False)
    sw2t_d = nc.declare_dram_parameter("sw2t", [IS, H], F32R, isOutput=False)
    xTs_d = nc.declare_dram_parameter("xTs", [H, TS], F32R, isOutput=False)
    esel_d = nc.declare_dram_parameter("esel", [P, 2, E], F32, isOutput=False)
    out_d = nc.declare_dram_parameter("out", [TS, H], F32, isOutput=True)
    routedp_d = nc.declare_dram_parameter("routedp", [T, H], F32, isOutput=True)
    debug = bool(int(os.environ.get("KMOE_DEBUG", "0")))
    if debug:
        dbg_s_d = nc.declare_dram_parameter("dbg_scores", [16, T], F32, isOutput=True)
        dbg_c_d = nc.declare_dram_parameter("dbg_C", [P, TT * E], F32, isOutput=True)
        dbg_y_d = [nc.declare_dram_parameter(f"dbg_y{e}", [P, KT_I, T], F32,
                                             isOutput=True) for e in range(E_PER_CORE)]
        dbg_w2_d = [nc.declare_dram_parameter(f"dbg_w2_{e}", [P, KT_I, H], F32,
                                              isOutput=True) for e in range(E_PER_CORE)]
        dbg_z_d = [nc.declare_dram_parameter(f"dbg_z{e}", [P, 512], F32,
                                             isOutput=True) for e in range(E_PER_CORE)]

    with tile.TileContext(nc) as tc:
        with tc.tile_pool(name="dram", bufs=1, space="DRAM") as dram, \
             tc.tile_pool(name="resident", bufs=1) as res:
            y_dram = [
                dram.tile([P, KT_I, T], F32R, name=f"y{e}_dram") for e in range(E_PER_CORE)
            ]

            # resident across phases
            C_sb = res.tile([P, TT, E], F32, name="C_sb")          # combine weights
            C2_sb = res.tile([P, TT, 2], F32, name="C2_sb")        # this core's 2 cols
            ident = res.tile([P, P], F32, name="ident")
            make_identity(nc, ident)

            # ---------------- Router (fp32) ----------------
            with tc.tile_pool(name="rt", bufs=2) as rt, \
                 tc.tile_pool(name="rt1", bufs=1) as rt1, \
                 tc.tile_pool(name="rtp", bufs=2, space="PSUM") as rtp:
                gw_sb = rt1.tile([P, KT_H, E], F32, name="gw_sb")
                nc.sync.dma_start(out=gw_sb, in_=_r3(gwT_d.ap()))
                biasb = rt1.tile([P, E], F32, name="biasb")
                nc.sync.dma_start(out=biasb, in_=biasb_d.ap())
                esel = rt1.tile([P, 2, E], F32, name="esel")
                nc.sync.dma_start(out=esel, in_=esel_d.ap())
                negbig = rt1.tile([P, E], F32, name="negbig")
                nc.vector.memset(negbig, -1e30)
                sT = rt1.tile([16, T], F32, name="sT")  # scores^T (expert-major)

                for n in range(NTOK):
                    xn = rt.tile([P, KT_H, TCH], F32R, name="xn_r", tag="xn_r")
                    nc.sync.dma_start(out=xn, in_=_r3(xT_d.ap())[:, :, n * TCH:(n + 1) * TCH])
                    xn32 = xn.bitcast(F32)
                    ps = rtp.tile([P, TCH], F32, name="ps_r", tag="ps_r")
                    # 4 col-group lanes x 4 accumulation rounds
                    for kt in range(KT_H):
                        lane, rnd = kt % 4, kt // 4
                        nc.tensor.matmul(
                            ps[32 * lane:32 * lane + 16, :],
                            gw_sb[:, kt, :], xn32[:, kt, :],
                            start=(rnd == 0), stop=(rnd == 3),
                            tile_position=(0, 32 * lane),
                        )
                    # merge the 4 lanes: PSUM -> SBUF (lane-aligned), then
                    # SBUF->SBUF DMAs to bring lanes 1-3 down to partitions 0-15
                    psb = rt.tile([P, TCH], F32, name="psb", tag="psb")
                    nc.vector.tensor_copy(psb, ps)
                    lanes = rt.tile([16, 3, TCH], F32, name="lanes", tag="lanes")
                    for l in range(1, 4):
                        nc.sync.dma_start(out=lanes[:, l - 1, :], in_=psb[32 * l:32 * l + 16, :])
                    acc = sT[:, n * TCH:(n + 1) * TCH]
                    nc.vector.tensor_tensor(acc, psb[0:16, :], lanes[:, 0, :], ALU.add)
                    nc.vector.tensor_tensor(acc, acc, lanes[:, 1, :], ALU.add)
                    nc.vector.tensor_tensor(acc, acc, lanes[:, 2, :], ALU.add)
                # sigmoid over full scores^T
                nc.scalar.activation(sT, sT, AF.Sigmoid)
                if debug:
                    nc.sync.dma_start(out=dbg_s_d.ap(), in_=sT)

                # transpose to token-major + grouped top-k per 128-token tile
                for tt in range(TT):
                    pst = rtp.tile([P, 16], F32, name="pst", tag="pst")
                    nc.tensor.transpose(pst, sT[:, tt * P:(tt + 1) * P], ident[:16, :16])
                    sc = rt.tile([P, E], F32, name="sc", tag="sc")   # raw scores
                    nc.vector.tensor_copy(sc, pst)
                    sel = rt.tile([P, E], F32, name="sel", tag="sel")
                    nc.vector.tensor_tensor(sel, sc, biasb, ALU.add)

                    # group score: sum of top-2 within each group of 4
                    a, b = sel[:, 0::4], sel[:, 1::4]
                    c_, d = sel[:, 2::4], sel[:, 3::4]
                    t4 = rt.tile([P, 6, G], F32, name="t4", tag="t4")
                    m1, n1, m2, n2, gs, tmp = (t4[:, j, :] for j in range(6))
                    nc.vector.tensor_tensor(m1, a, b, ALU.max)
                    nc.vector.tensor_tensor(n1, a, b, ALU.min)
                    nc.vector.tensor_tensor(m2, c_, d, ALU.max)
                    nc.vector.tensor_tensor(n2, c_, d, ALU.min)
                    nc.vector.tensor_tensor(gs, m1, m2, ALU.add)
                    nc.vector.tensor_tensor(tmp, m1, n1, ALU.add)
                    nc.vector.tensor_tensor(gs, gs, tmp, ALU.max)
                    nc.vector.tensor_tensor(tmp, m2, n2, ALU.add)
                    nc.vector.tensor_tensor(gs, gs, tmp, ALU.max)

                    gs8 = rt.tile([P, 8], F32, name="gs8", tag="gs8")
                    nc.vector.memset(gs8[:, G:], -1e30)
                    nc.vector.tensor_copy(gs8[:, :G], gs)
                    g8 = rt.tile([P, 8], F32, name="g8", tag="g8")
                    nc.vector.max(g8, gs8)
                    gmask = rt.tile([P, G], F32, name="gmask", tag="gmask")
                    nc.vector.tensor_scalar(gmask, gs, g8[:, 1:2], None, ALU.is_ge)
                    emask = rt.tile([P, E], F32, name="emask", tag="emask")
                    for j in range(4):
                        nc.vector.tensor_copy(emask[:, j::4], gmask)
                    # masked = sel + (emask-1)*1e30  (exact for selected entries)
                    masked = rt.tile([P, E], F32, name="masked", tag="masked")
                    em1 = rt.tile([P, E], F32, name="em1", tag="em1")
                    nc.vector.tensor_scalar_add(em1, emask, -1.0)
                    nc.vector.scalar_tensor_tensor(masked, em1, 1e30, sel,
                                                   ALU.mult, ALU.add)
                    m8 = rt.tile([P, 8], F32, name="m8", tag="m8")
                    nc.vector.max(m8, masked)
                    selm = rt.tile([P, E], F32, name="selm", tag="selm")
                    nc.vector.tensor_scalar(selm, masked, m8[:, 3:4], None, ALU.is_ge)
                    cw = rt.tile([P, E], F32, name="cw", tag="cw")
                    nc.vector.tensor_tensor(cw, sc, selm, ALU.mult)
                    den = rt.tile([P, 2], F32, name="den", tag="den")
                    nc.vector.reduce_sum(den[:, 0:1], cw, AX)
                    nc.vector.tensor_scalar_add(den[:, 0:1], den[:, 0:1], 1e-20)
                    nc.vector.reciprocal(den[:, 1:2], den[:, 0:1])
                    nc.vector.tensor_scalar_mul(den[:, 1:2], den[:, 1:2], ROUTED_SCALE)
                    nc.vector.tensor_scalar_mul(C_sb[:, tt, :], cw, den[:, 1:2])
                    esm = rt.tile([P, 2, E], F32, name="esm", tag="esm")
                    nc.vector.tensor_tensor(esm[:, 0, :], C_sb[:, tt, :], esel[:, 0, :], ALU.mult)
                    nc.vector.tensor_tensor(esm[:, 1, :], C_sb[:, tt, :], esel[:, 1, :], ALU.mult)
                    nc.vector.reduce_sum(C2_sb[:, tt, 0:1], esm[:, 0, :], AX)
                    nc.vector.reduce_sum(C2_sb[:, tt, 1:2], esm[:, 1, :], AX)

            # ---------------- Pass A: gate/up proj per expert ----------------
            with tc.tile_pool(name="aw", bufs=2) as aw, \
                 tc.tile_pool(name="ax", bufs=2) as ax_, \
                 tc.tile_pool(name="ay", bufs=3) as ay, \
                 tc.tile_pool(name="ap_ps", bufs=2, space="PSUM") as aps:
                for e in range(E_PER_CORE):
                    for h in range(IH):
                        w1h = aw.tile([P, KT_H, IHW], F32R, name="w1h", tag="w1h")
                        w3h = aw.tile([P, KT_H, IHW], F32R, name="w3h", tag="w3h")
                        isl = slice(h * IHW, (h + 1) * IHW)
                        nc.sync.dma_start(out=w1h, in_=_r3(w1t_d.ap()[e])[:, :, isl])
                        nc.sync.dma_start(out=w3h, in_=_r3(w3t_d.ap()[e])[:, :, isl])
                        for n in range(NTOK):
                            xn = ax_.tile([P, KT_H, TCH], F32R, name="xn_a", tag="xn_a")
                            nc.sync.dma_start(
                                out=xn, in_=_r3(xT_d.ap())[:, :, n * TCH:(n + 1) * TCH])
                            for m in range(IHW // P):
                                msl = slice(m * P, (m + 1) * P)
                                pg = aps.tile([P, TCH], F32, name="pg", tag="pg")
                                pu = aps.tile([P, TCH], F32, name="pu", tag="pu")
                                for kt in range(KT_H):
                                    nc.tensor.matmul(pg, w1h[:, kt, msl], xn[:, kt, :],
                                                     start=(kt == 0), stop=(kt == KT_H - 1))
                                for kt in range(KT_H):
                                    nc.tensor.matmul(pu, w3h[:, kt, msl], xn[:, kt, :],
                                                     start=(kt == 0), stop=(kt == KT_H - 1))
                                sg = ay.tile([P, TCH], F32, name="sg", tag="sg")
                                nc.scalar.activation(sg, pg, AF.Silu)
                                y = ay.tile([P, TCH], F32R, name="y", tag="y")
                                nc.vector.tensor_tensor(y, sg, pu, ALU.mult)
                                nc.sync.dma_start(
                                    out=y_dram[e][:, h * (IHW // P) + m,
                                                  n * TCH:(n + 1) * TCH],
                                    in_=y)

            tc.strict_bb_all_engine_barrier()
            # ---------------- Pass C: down proj + combine ----------------
            with tc.tile_pool(name="cw2", bufs=1) as cw2, \
                 tc.tile_pool(name="cy", bufs=2) as cy, \
                 tc.tile_pool(name="co", bufs=3) as co, \
                 tc.tile_pool(name="cps", bufs=2, space="PSUM") as cps:
                w2 = [cw2.tile([P, KT_I, H], F32R, name=f"w2_{e}") for e in range(E_PER_CORE)]
                for e in range(E_PER_CORE):
                    nc.sync.dma_start(out=w2[e], in_=_r3(w2t_d.ap()[e]))
                    if debug:
                        nc.sync.dma_start(out=dbg_w2_d[e].ap(), in_=w2[e].bitcast(F32))
                for tt in range(TT):
                    tsl = slice(tt * P, (tt + 1) * P)
                    yt = []
                    for e in range(E_PER_CORE):
                        yte = cy.tile([P, KT_I, P], F32R, name="yt", tag=f"yt{e}")
                        nc.sync.dma_start(out=yte, in_=y_dram[e][:, :, tsl])
                        yt.append(yte)
                    for hc in range(HC):
                        hsl = slice(hc * 512, (hc + 1) * 512)
                        pz0 = cps.tile([P, 512], F32, name="pz0", tag="pz0")
                        pz1 = cps.tile([P, 512], F32, name="pz1", tag="pz1")
                        for ki in range(KT_I):
                            nc.tensor.matmul(pz0, yt[0][:, ki, :], w2[0][:, ki, hsl],
                                             start=(ki == 0), stop=(ki == KT_I - 1))
                        for ki in range(KT_I):
                            nc.tensor.matmul(pz1, yt[1][:, ki, :], w2[1][:, ki, hsl],
                                             start=(ki == 0), stop=(ki == KT_I - 1))
                        if debug and tt == 0 and hc == 0:
                            for e, pz in ((0, pz0), (1, pz1)):
                                zdbg = co.tile([P, 512], F32, name="zdbg", tag="zdbg")
                                nc.vector.tensor_copy(zdbg, pz)
                                nc.sync.dma_start(out=dbg_z_d[e].ap(), in_=zdbg)
                        zc = co.tile([P, 512], F32, name="zc", tag="zc")
                        nc.vector.tensor_scalar_mul(zc, pz0, C2_sb[:, tt, 0:1])
                        nc.vector.scalar_tensor_tensor(
                            zc, pz1, C2_sb[:, tt, 1:2], zc, ALU.mult, ALU.add)
                        nc.sync.dma_start(out=routedp_d.ap()[tsl, hsl], in_=zc)

            if debug:
                nc.sync.dma_start(out=dbg_c_d.ap(),
                                  in_=C_sb.rearrange("p a b -> p (a b)"))
                pass
                for e in range(E_PER_CORE):
                    nc.sync.dma_start(out=dbg_y_d[e].ap(),
                                      in_=y_dram[e][:, :, :].bitcast(F32))

            # ---------------- Shared expert on own 256-token slice ----------------
            with tc.tile_pool(name="sres", bufs=1) as sres, \
                 tc.tile_pool(name="sy", bufs=2) as sy, \
                 tc.tile_pool(name="so", bufs=3) as so, \
                 tc.tile_pool(name="sps", bufs=2, space="PSUM") as sps:
                ys = sres.tile([P, KT_I, TS], F32R, name="ys")
                with tc.tile_pool(name="swa", bufs=1) as swa:
                    xs = swa.tile([P, KT_H, TS], F32R, name="xs")
                    nc.sync.dma_start(out=xs, in_=_r3(xTs_d.ap()))
                    for h in range(IH):
                        sw1h = swa.tile([P, KT_H, IHW], F32R, name="sw1h", tag="sw1h")
                        sw3h = swa.tile([P, KT_H, IHW], F32R, name="sw3h", tag="sw3h")
                        isl = slice(h * IHW, (h + 1) * IHW)
                        nc.sync.dma_start(out=sw1h, in_=_r3(sw1t_d.ap())[:, :, isl])
                        nc.sync.dma_start(out=sw3h, in_=_r3(sw3t_d.ap())[:, :, isl])
                        for m in range(IHW // P):
                            mi = h * (IHW // P) + m
                            msl = slice(m * P, (m + 1) * P)
                            pg = sps.tile([P, TS], F32, name="spg", tag="spg")
                            pu = sps.tile([P, TS], F32, name="spu", tag="spu")
                            for kt in range(KT_H):
                                nc.tensor.matmul(pg, sw1h[:, kt, msl], xs[:, kt, :],
                                                 start=(kt == 0), stop=(kt == KT_H - 1))
                            for kt in range(KT_H):
                                nc.tensor.matmul(pu, sw3h[:, kt, msl], xs[:, kt, :],
                                                 start=(kt == 0), stop=(kt == KT_H - 1))
                            sg = sy.tile([P, TS], F32, name="ssg", tag="ssg")
                            nc.scalar.activation(sg, pg, AF.Silu)
                            nc.vector.tensor_tensor(ys[:, mi, :], sg, pu, ALU.mult)
                sw2 = sres.tile([P, KT_I, H], F32R, name="sw2")
                nc.sync.dma_start(out=sw2, in_=_r3(sw2t_d.ap()))
                for tt in range(TS // P):
                    tsl = slice(tt * P, (tt + 1) * P)
                    for hc in range(HC):
                        hsl = slice(hc * 512, (hc + 1) * 512)
                        pz = sps.tile([P, 512], F32, name="spz", tag="spz")
                        for ki in range(KT_I):
                            nc.tensor.matmul(pz, ys[:, ki, tsl], sw2[:, ki, hsl],
                                             start=(ki == 0), stop=(ki == KT_I - 1))
                        ot = so.tile([P, 512], F32, name="ot", tag="ot")
                        nc.vector.tensor_copy(ot, pz)
                        nc.sync.dma_start(out=out_d.ap()[tsl, hsl], in_=ot)

    nc.compile()
    return nc


_NC_CACHE = None


def _get_nc():
    global _NC_CACHE
    if _NC_CACHE is None:
        _NC_CACHE = build_nc()
    return _NC_CACHE


def esel_host(c):
    m = np.zeros((P, 2, E), np.float32)
    m[:, 0, 2 * c] = 1.0
    m[:, 1, 2 * c + 1] = 1.0
    return m


def kernel(hidden_states, gate_w, expert_bias, w1, w3, w2, sw1, sw3, sw2):
    hidden_states = np.ascontiguousarray(hidden_states, dtype=np.float32)
    xT = np.ascontiguousarray(hidden_states.T)
    gwT = np.ascontiguousarray(gate_w.T.astype(np.float32))
    biasb = np.ascontiguousarray(
        np.broadcast_to(expert_bias.astype(np.float32)[None, :], (P, E)))
    w1t = np.ascontiguousarray(np.transpose(w1.astype(np.float32), (0, 2, 1)))
    w3t = np.ascontiguousarray(np.transpose(w3.astype(np.float32), (0, 2, 1)))
    w2t = np.ascontiguousarray(np.transpose(w2.astype(np.float32), (0, 2, 1)))
    sw1t = np.ascontiguousarray(sw1.astype(np.float32).T)
    sw3t = np.ascontiguousarray(sw3.astype(np.float32).T)
    sw2t = np.ascontiguousarray(sw2.astype(np.float32).T)

    in_maps = []
    for c in range(N_CORES):
        es = slice(E_PER_CORE * c, E_PER_CORE * (c + 1))
        in_maps.append({
            "xT": xT,
            "gwT": gwT,
            "biasb": biasb,
            "w1t": w1t[es],
            "w3t": w3t[es],
            "w2t": w2t[es],
            "sw1t": sw1t,
            "sw3t": sw3t,
            "sw2t": sw2t,
            "xTs": np.ascontiguousarray(xT[:, TS * c:TS * (c + 1)]),
            "esel": esel_host(c),
        })

    nc = _get_nc()
    res = run_bass_kernel_spmd(nc, in_maps, list(range(N_CORES)))
    out = res.results[0]["routedp"].copy()
    for c in range(1, N_CORES):
        out += res.results[c]["routedp"]
    for c in range(N_CORES):
        out[TS * c:TS * (c + 1)] += res.results[c]["out"]
    kernel.last_result = res
    return out.astype(np.float32)
